# revision 7
# baseline (speedup 1.0000x reference)
"""Trainium2 Bass kernel for ConditionalAffineCoupling (dense MLP coupling layer).

Computation (per row of the batch):
    x_masked  = x[:, 0::2]            # [B, 256]
    x_unmask  = x[:, 1::2]            # [B, 256]
    st_in     = concat([x_masked, context], -1)      # [B, 384]
    h         = relu(st_in @ W1 + b1)                # [B, 2048]
    st        = h @ W2 + b2                          # [B, 512]
    s         = tanh(st[:, :256]) * 5.0
    y_odd     = x_unmask * exp(s) + st[:, 256:]
    y         = x with odd cols replaced by y_odd
    log_det   = s.sum(-1)

Strategy: data-parallel over 8 NeuronCores (batch sharded, weights
replicated). Per core, 512-row tiles:
  - PE transposes build st_in^T in SBUF (fp32, exact),
  - layer 1 in float32r (full PE rate at N=512): h^T = W1^T-slices
    (stationary) x st_in^T, bias+ReLU fused on ScalarE during the
    PSUM->SBUF drain,
  - layer 2 produces st in natural [row, feature] layout: lhsT = h^T
    sub-tile, rhs = W2 (resident); b2 folded in via a K=1 matmul
    against a constant-ones row,
  - epilogue: tanh on ScalarE with accum_out accumulating
    sum(tanh) per row (log_det = 5 * that), exp(5*tanh) on ScalarE,
    then two in-place DVE ops write y's odd columns straight into the
    x tile, which is DMAed out as full y rows.
"""

import sys

sys.path.insert(0, "/opt/trn_rl_repo")

import numpy as np

import concourse.mybir as mybir
import concourse.tile as tile
from concourse import bacc
from concourse.bass_utils import run_bass_kernel_spmd
from concourse.masks import make_identity

B, DIM, CTX, HID = 131072, 512, 128, 2048
DU = DIM // 2  # 256
S_MAX = 5.0
N_CORES = 8
BS = B // N_CORES  # 16384 rows per core
TR = 512  # rows per tile
NT = BS // TR  # 32 tiles
SUB = TR // 128  # 4 sub-tiles of 128 rows
KT1 = (DIM // 2 + CTX) // 128  # 3 contraction tiles, layer 1
KT2 = HID // 128  # 16 contraction tiles, layer 2

F32 = mybir.dt.float32
F32R = mybir.dt.float32r
AF = mybir.ActivationFunctionType

_CACHE = {}


def build_nc(repeat=1):
    nc = bacc.Bacc("TRN2", target_bir_lowering=False, debug=False)
    x_d = nc.dram_tensor("x", [BS, DIM], F32, kind="ExternalInput")
    c_d = nc.dram_tensor("context", [BS, CTX], F32, kind="ExternalInput")
    w1_d = nc.dram_tensor("W1", [DIM // 2 + CTX, HID], F32, kind="ExternalInput")
    b1_d = nc.dram_tensor("b1", [HID], F32, kind="ExternalInput")
    w2_d = nc.dram_tensor("W2", [HID, 2 * DU], F32, kind="ExternalInput")
    b2_d = nc.dram_tensor("b2", [2 * DU], F32, kind="ExternalInput")
    y_d = nc.dram_tensor("y", [BS, DIM], F32, kind="ExternalOutput")
    ld_d = nc.dram_tensor("log_det", [BS], F32, kind="ExternalOutput")

    # [NT, 128, SUB, DIM]: row r = (t*SUB + j)*128 + p
    x4 = x_d.ap().rearrange("(t j p) d -> t p j d", p=128, j=SUB)
    c4 = c_d.ap().rearrange("(t j p) d -> t p j d", p=128, j=SUB)
    y4 = y_d.ap().rearrange("(t j p) d -> t p j d", p=128, j=SUB)
    # weights: contraction index k = kt*128 + p
    w1r = w1_d.ap().bitcast(F32R).rearrange("(kt p) n -> p kt n", p=128)
    w2r = w2_d.ap().bitcast(F32R).rearrange("(kt p) n -> p kt n", p=128)
    b1v = b1_d.ap().rearrange("(t p) -> p t", p=128)  # [128, KT2]
    b2r = b2_d.ap().bitcast(F32R).rearrange("(o n) -> o n", o=1)  # [1, 512]
    ld2 = ld_d.ap().rearrange("(c p) -> c p", p=128)  # [128, 128]

    with tile.TileContext(nc) as tc:
        with (
            tc.tile_pool(name="const", bufs=1) as constp,
            tc.tile_pool(name="wpool", bufs=1) as wp,
            tc.tile_pool(name="xpool", bufs=2) as xp,
            tc.tile_pool(name="stpool", bufs=2) as stp,
            tc.tile_pool(name="hpool", bufs=1) as hp,
            tc.tile_pool(name="epi", bufs=3) as ep,
            tc.tile_pool(name="ldp", bufs=1) as ldp,
            tc.tile_pool(name="pstr", bufs=2, space="PSUM") as pst,
            tc.tile_pool(name="ps1", bufs=3, space="PSUM") as ps1,
            tc.tile_pool(name="ps2", bufs=2, space="PSUM") as ps2,
        ):
            ident = constp.tile([128, 128], F32)
            make_identity(nc, ident[:])
            ones_f = constp.tile([1, 128], F32)
            nc.gpsimd.memset(ones_f[:], 1.0)
            ones_r = constp.tile([1, 128], F32R)
            nc.scalar.copy(ones_r[:], ones_f[:])
            zeros = constp.tile([128, TR], F32)
            nc.gpsimd.memset(zeros[:], 0.0)

            w1sb = wp.tile([128, KT1, HID], F32R)
            nc.sync.dma_start(w1sb[:], w1r)
            w2sb = wp.tile([128, KT2, 2 * DU], F32R)
            nc.sync.dma_start(w2sb[:], w2r)
            b1sb = wp.tile([128, KT2], F32)
            nc.sync.dma_start(b1sb[:], b1v)
            b2sb = wp.tile([1, 2 * DU], F32R)
            nc.sync.dma_start(b2sb[:], b2r)

            def body(rep=0):
                # per-row sum(tanh); column c = global 128-row sub-tile index
                ld_all = ldp.tile(
                    [128, NT * SUB], F32, name=f"ld_all{rep}", tag="ld_all"
                )

                for t in range(NT):
                    xt = xp.tile([128, SUB, DIM], F32, name=f"xt{t}", tag="xt")
                    nc.sync.dma_start(xt[:], x4[t])
                    ct = xp.tile([128, SUB, CTX], F32, name=f"ct{t}", tag="ct")
                    nc.sync.dma_start(ct[:], c4[t])

                    # st_in^T [384, TR] as [128, KT1, TR] (k = kt*128 + p)
                    stT = stp.tile([128, KT1, TR], F32R, name=f"stT{t}", tag="stT")
                    for j in range(SUB):
                        xj = xt[:, j].rearrange("p (d two) -> p two d", two=2)
                        for ft in range(2):
                            tp = pst.tile(
                                [128, 128], F32, name=f"tp{t}_{j}_{ft}", tag="tp"
                            )
                            nc.tensor.transpose(
                                tp[:], xj[:, 0, ft * 128 : (ft + 1) * 128], ident[:]
                            )
                            nc.vector.tensor_copy(
                                stT[:, ft, j * 128 : (j + 1) * 128], tp[:]
                            )
                        tpc = pst.tile([128, 128], F32, name=f"tpc{t}_{j}", tag="tp")
                        nc.tensor.transpose(tpc[:], ct[:, j], ident[:])
                        nc.vector.tensor_copy(
                            stT[:, 2, j * 128 : (j + 1) * 128], tpc[:]
                        )

                    # layer 1: h^T [HID, TR] as [128, KT2, TR]
                    hT = hp.tile([128, KT2, TR], F32R, name=f"hT{t}", tag="hT")
                    for ht in range(KT2):
                        ph = ps1.tile([128, TR], F32, name=f"ph{t}_{ht}", tag="ph")
                        for kt in range(KT1):
                            nc.tensor.matmul(
                                ph[:],
                                w1sb[:, kt, ht * 128 : (ht + 1) * 128],
                                stT[:, kt, :],
                                start=(kt == 0),
                                stop=(kt == KT1 - 1),
                            )
                        # relu(psum + b1) -> f32r on DVE
                        nc.vector.scalar_tensor_tensor(
                            hT[:, ht, :],
                            ph[:],
                            b1sb[:, ht : ht + 1],
                            zeros[:],
                            op0=mybir.AluOpType.add,
                            op1=mybir.AluOpType.max,
                        )

                    # layer 2 + epilogue per 128-row sub-tile
                    for j in range(SUB):
                        p2 = ps2.tile([128, 2 * DU], F32, name=f"p2_{t}_{j}", tag="p2")
                        nc.tensor.matmul(
                            p2[:], ones_r[:], b2sb[:], start=True, stop=False
                        )
                        for kt in range(KT2):
                            nc.tensor.matmul(
                                p2[:],
                                hT[:, kt, j * 128 : (j + 1) * 128],
                                w2sb[:, kt, :],
                                start=False,
                                stop=(kt == KT2 - 1),
                            )
                        tanh_s = ep.tile(
                            [128, DU], F32, name=f"tanh{t}_{j}", tag="tanh"
                        )
                        col = t * SUB + j
                        nc.scalar.activation(
                            tanh_s[:],
                            p2[:, 0:DU],
                            AF.Tanh,
                            accum_out=ld_all[:, col : col + 1],
                        )
                        exp_s = ep.tile([128, DU], F32, name=f"exp{t}_{j}", tag="exp")
                        nc.scalar.activation(exp_s[:], tanh_s[:], AF.Exp, scale=S_MAX)
                        xj = xt[:, j].rearrange("p (d two) -> p two d", two=2)
                        nc.vector.tensor_mul(xj[:, 1, :], xj[:, 1, :], exp_s[:])
                        nc.vector.tensor_add(
                            xj[:, 1, :], xj[:, 1, :], p2[:, DU : 2 * DU]
                        )
                        nc.sync.dma_start(y4[t, :, j], xt[:, j])

                # log_det = 5 * sum(tanh): transpose ld_all then scale on drain
                ldps = pst.tile([128, 128], F32, name="ldps", tag="tp")
                nc.tensor.transpose(ldps[:], ld_all[:], ident[:])
                ld_out = ldp.tile([128, 128], F32, name=f"ld_out{rep}", tag="ld_out")
                nc.scalar.mul(ld_out[:], ldps[:], S_MAX)
                nc.sync.dma_start(ld2, ld_out[:])

            if repeat == 1:
                body()
            else:
                with tc.For_i(0, repeat, 1):
                    body()

    nc.compile()
    return nc


def make_in_maps(inputs):
    x = np.ascontiguousarray(np.asarray(inputs["x"], dtype=np.float32))
    ctx = np.ascontiguousarray(np.asarray(inputs["context"], dtype=np.float32))
    w1 = np.ascontiguousarray(np.asarray(inputs["W1"], dtype=np.float32))
    b1 = np.ascontiguousarray(np.asarray(inputs["b1"], dtype=np.float32))
    w2 = np.ascontiguousarray(np.asarray(inputs["W2"], dtype=np.float32))
    b2 = np.ascontiguousarray(np.asarray(inputs["b2"], dtype=np.float32))

    in_maps = []
    for i in range(N_CORES):
        sl = slice(i * BS, (i + 1) * BS)
        in_maps.append(
            {
                "x": np.ascontiguousarray(x[sl]),
                "context": np.ascontiguousarray(ctx[sl]),
                "W1": w1,
                "b1": b1,
                "W2": w2,
                "b2": b2,
            }
        )
    return in_maps


def _run(inputs, trace=False):
    if "nc" not in _CACHE:
        _CACHE["nc"] = build_nc()
    nc = _CACHE["nc"]
    in_maps = make_in_maps(inputs)
    res = run_bass_kernel_spmd(nc, in_maps, core_ids=list(range(N_CORES)), trace=trace)
    y = np.concatenate([r["y"] for r in res.results], axis=0)
    ld = np.concatenate([r["log_det"] for r in res.results], axis=0)
    return (y, ld), res


def kernel(**inputs):
    out, _ = _run(inputs, trace=False)
    return out


# revision 10
# speedup vs baseline: 1.0011x; 1.0011x over previous
"""Trainium2 Bass kernel for ConditionalAffineCoupling (dense MLP coupling layer).

Computation (per row of the batch):
    x_masked  = x[:, 0::2]            # [B, 256]
    x_unmask  = x[:, 1::2]            # [B, 256]
    st_in     = concat([x_masked, context], -1)      # [B, 384]
    h         = relu(st_in @ W1 + b1)                # [B, 2048]
    st        = h @ W2 + b2                          # [B, 512]
    s         = tanh(st[:, :256]) * 5.0
    y_odd     = x_unmask * exp(s) + st[:, 256:]
    y         = x with odd cols replaced by y_odd
    log_det   = s.sum(-1)

Strategy: data-parallel over 8 NeuronCores (batch sharded, weights
replicated). Per core, 512-row tiles:
  - PE transposes build st_in^T in SBUF (fp32, exact),
  - layer 1 in float32r (full PE rate at N=512): h^T = W1^T-slices
    (stationary) x st_in^T, bias+ReLU fused on ScalarE during the
    PSUM->SBUF drain,
  - layer 2 produces st in natural [row, feature] layout: lhsT = h^T
    sub-tile, rhs = W2 (resident); b2 folded in via a K=1 matmul
    against a constant-ones row,
  - epilogue: tanh on ScalarE with accum_out accumulating
    sum(tanh) per row (log_det = 5 * that), exp(5*tanh) on ScalarE,
    then two in-place DVE ops write y's odd columns straight into the
    x tile, which is DMAed out as full y rows.
"""

import sys

sys.path.insert(0, "/opt/trn_rl_repo")

import numpy as np

import concourse.mybir as mybir
import concourse.tile as tile
from concourse import bacc
from concourse.bass_utils import run_bass_kernel_spmd
from concourse.masks import make_identity

B, DIM, CTX, HID = 131072, 512, 128, 2048
DU = DIM // 2  # 256
S_MAX = 5.0
N_CORES = 8
BS = B // N_CORES  # 16384 rows per core
TR = 512  # rows per tile
NT = BS // TR  # 32 tiles
SUB = TR // 128  # 4 sub-tiles of 128 rows
KT1 = (DIM // 2 + CTX) // 128  # 3 contraction tiles, layer 1
KT2 = HID // 128  # 16 contraction tiles, layer 2

F32 = mybir.dt.float32
F32R = mybir.dt.float32r
AF = mybir.ActivationFunctionType

_CACHE = {}


def build_nc(repeat=1):
    nc = bacc.Bacc("TRN2", target_bir_lowering=False, debug=False)
    x_d = nc.dram_tensor("x", [BS, DIM], F32, kind="ExternalInput")
    c_d = nc.dram_tensor("context", [BS, CTX], F32, kind="ExternalInput")
    w1_d = nc.dram_tensor("W1", [DIM // 2 + CTX, HID], F32, kind="ExternalInput")
    b1_d = nc.dram_tensor("b1", [HID], F32, kind="ExternalInput")
    w2_d = nc.dram_tensor("W2", [HID, 2 * DU], F32, kind="ExternalInput")
    b2_d = nc.dram_tensor("b2", [2 * DU], F32, kind="ExternalInput")
    y_d = nc.dram_tensor("y", [BS, DIM], F32, kind="ExternalOutput")
    ld_d = nc.dram_tensor("log_det", [BS], F32, kind="ExternalOutput")

    # [NT, 128, SUB, DIM]: row r = (t*SUB + j)*128 + p
    x4 = x_d.ap().rearrange("(t j p) d -> t p j d", p=128, j=SUB)
    c4 = c_d.ap().rearrange("(t j p) d -> t p j d", p=128, j=SUB)
    y4 = y_d.ap().rearrange("(t j p) d -> t p j d", p=128, j=SUB)
    # weights: contraction index k = kt*128 + p
    w1r = w1_d.ap().bitcast(F32R).rearrange("(kt p) n -> p kt n", p=128)
    w2r = w2_d.ap().bitcast(F32R).rearrange("(kt p) n -> p kt n", p=128)
    b1v = b1_d.ap().rearrange("(t p) -> p t", p=128)  # [128, KT2]
    b2r = b2_d.ap().bitcast(F32R).rearrange("(o n) -> o n", o=1)  # [1, 512]
    ld2 = ld_d.ap().rearrange("(c p) -> c p", p=128)  # [128, 128]

    with tile.TileContext(nc) as tc:
        with (
            tc.tile_pool(name="const", bufs=1) as constp,
            tc.tile_pool(name="wpool", bufs=1) as wp,
            tc.tile_pool(name="xpool", bufs=2) as xp,
            tc.tile_pool(name="stpool", bufs=2) as stp,
            tc.tile_pool(name="hpool", bufs=1) as hp,
            tc.tile_pool(name="epi", bufs=3) as ep,
            tc.tile_pool(name="ldp", bufs=1) as ldp,
            tc.tile_pool(name="pstr", bufs=2, space="PSUM") as pst,
            tc.tile_pool(name="ps1", bufs=3, space="PSUM") as ps1,
            tc.tile_pool(name="ps2", bufs=2, space="PSUM") as ps2,
        ):
            ident = constp.tile([128, 128], F32)
            make_identity(nc, ident[:])
            ones_f = constp.tile([1, 128], F32)
            nc.gpsimd.memset(ones_f[:], 1.0)
            ones_r = constp.tile([1, 128], F32R)
            nc.scalar.copy(ones_r[:], ones_f[:])
            zeros = constp.tile([128, TR], F32)
            nc.gpsimd.memset(zeros[:], 0.0)

            # weights go via SWDGE (gpsimd) so the sync HWDGE queue is free
            # for the first x/context tiles; b1/b2 are tiny, send first.
            b1sb = wp.tile([128, KT2], F32)
            nc.gpsimd.dma_start(b1sb[:], b1v)
            b2sb = wp.tile([1, 2 * DU], F32R)
            nc.gpsimd.dma_start(b2sb[:], b2r)
            w1sb = wp.tile([128, KT1, HID], F32R)
            nc.gpsimd.dma_start(w1sb[:], w1r)
            w2sb = wp.tile([128, KT2, 2 * DU], F32R)
            nc.gpsimd.dma_start(w2sb[:], w2r)

            def body(rep=0):
                # per-row sum(tanh); column c = global 128-row sub-tile index
                ld_all = ldp.tile(
                    [128, NT * SUB], F32, name=f"ld_all{rep}", tag="ld_all"
                )

                for t in range(NT):
                    xt = xp.tile([128, SUB, DIM], F32, name=f"xt{t}", tag="xt")
                    nc.sync.dma_start(xt[:], x4[t])
                    ct = xp.tile([128, SUB, CTX], F32, name=f"ct{t}", tag="ct")
                    nc.sync.dma_start(ct[:], c4[t])

                    # st_in^T [384, TR] as [128, KT1, TR] (k = kt*128 + p)
                    stT = stp.tile([128, KT1, TR], F32R, name=f"stT{t}", tag="stT")
                    for j in range(SUB):
                        xj = xt[:, j].rearrange("p (d two) -> p two d", two=2)
                        for ft in range(2):
                            tp = pst.tile(
                                [128, 128], F32, name=f"tp{t}_{j}_{ft}", tag="tp"
                            )
                            nc.tensor.transpose(
                                tp[:], xj[:, 0, ft * 128 : (ft + 1) * 128], ident[:]
                            )
                            nc.vector.tensor_copy(
                                stT[:, ft, j * 128 : (j + 1) * 128], tp[:]
                            )
                        tpc = pst.tile([128, 128], F32, name=f"tpc{t}_{j}", tag="tp")
                        nc.tensor.transpose(tpc[:], ct[:, j], ident[:])
                        nc.vector.tensor_copy(
                            stT[:, 2, j * 128 : (j + 1) * 128], tpc[:]
                        )

                    # layer 1: h^T [HID, TR] as [128, KT2, TR]
                    hT = hp.tile([128, KT2, TR], F32R, name=f"hT{t}", tag="hT")
                    for ht in range(KT2):
                        ph = ps1.tile([128, TR], F32, name=f"ph{t}_{ht}", tag="ph")
                        for kt in range(KT1):
                            nc.tensor.matmul(
                                ph[:],
                                w1sb[:, kt, ht * 128 : (ht + 1) * 128],
                                stT[:, kt, :],
                                start=(kt == 0),
                                stop=(kt == KT1 - 1),
                            )
                        # relu(psum + b1) -> f32r on DVE
                        nc.vector.scalar_tensor_tensor(
                            hT[:, ht, :],
                            ph[:],
                            b1sb[:, ht : ht + 1],
                            zeros[:],
                            op0=mybir.AluOpType.add,
                            op1=mybir.AluOpType.max,
                        )

                    # layer 2 + epilogue per 128-row sub-tile
                    for j in range(SUB):
                        p2 = ps2.tile([128, 2 * DU], F32, name=f"p2_{t}_{j}", tag="p2")
                        nc.tensor.matmul(
                            p2[:], ones_r[:], b2sb[:], start=True, stop=False
                        )
                        for kt in range(KT2):
                            nc.tensor.matmul(
                                p2[:],
                                hT[:, kt, j * 128 : (j + 1) * 128],
                                w2sb[:, kt, :],
                                start=False,
                                stop=(kt == KT2 - 1),
                            )
                        tanh_s = ep.tile(
                            [128, DU], F32, name=f"tanh{t}_{j}", tag="tanh"
                        )
                        col = t * SUB + j
                        nc.scalar.activation(
                            tanh_s[:],
                            p2[:, 0:DU],
                            AF.Tanh,
                            accum_out=ld_all[:, col : col + 1],
                        )
                        exp_s = ep.tile([128, DU], F32, name=f"exp{t}_{j}", tag="exp")
                        nc.scalar.activation(exp_s[:], tanh_s[:], AF.Exp, scale=S_MAX)
                        xj = xt[:, j].rearrange("p (d two) -> p two d", two=2)
                        nc.vector.tensor_mul(xj[:, 1, :], xj[:, 1, :], exp_s[:])
                        nc.vector.tensor_add(
                            xj[:, 1, :], xj[:, 1, :], p2[:, DU : 2 * DU]
                        )
                        # y out via the ACT HWDGE queue (inputs own sync's)
                        nc.scalar.dma_start(y4[t, :, j], xt[:, j])

                    # log_det = 5 * sum(tanh): drain every 8 tiles so the
                    # tail transpose isn't serialized at the very end
                    if (t + 1) % 8 == 0:
                        c = (t + 1) // 8 - 1
                        cols = slice(32 * c, 32 * (c + 1))
                        ldps = pst.tile(
                            [32, 128], F32, name=f"ldps{c}", tag="ldps", bufs=1
                        )
                        nc.tensor.transpose(ldps[:], ld_all[:, cols], ident[:])
                        ld_out = ldp.tile(
                            [32, 128], F32, name=f"ld_out{rep}_{c}", tag="ld_out"
                        )
                        nc.scalar.mul(ld_out[:], ldps[:], S_MAX)
                        nc.scalar.dma_start(ld2[cols], ld_out[:])

            if repeat == 1:
                body()
            else:
                with tc.For_i(0, repeat, 1):
                    body()

    nc.compile()
    return nc


def make_in_maps(inputs):
    x = np.ascontiguousarray(np.asarray(inputs["x"], dtype=np.float32))
    ctx = np.ascontiguousarray(np.asarray(inputs["context"], dtype=np.float32))
    w1 = np.ascontiguousarray(np.asarray(inputs["W1"], dtype=np.float32))
    b1 = np.ascontiguousarray(np.asarray(inputs["b1"], dtype=np.float32))
    w2 = np.ascontiguousarray(np.asarray(inputs["W2"], dtype=np.float32))
    b2 = np.ascontiguousarray(np.asarray(inputs["b2"], dtype=np.float32))

    in_maps = []
    for i in range(N_CORES):
        sl = slice(i * BS, (i + 1) * BS)
        in_maps.append(
            {
                "x": np.ascontiguousarray(x[sl]),
                "context": np.ascontiguousarray(ctx[sl]),
                "W1": w1,
                "b1": b1,
                "W2": w2,
                "b2": b2,
            }
        )
    return in_maps


def _run(inputs, trace=False):
    if "nc" not in _CACHE:
        _CACHE["nc"] = build_nc()
    nc = _CACHE["nc"]
    in_maps = make_in_maps(inputs)
    res = run_bass_kernel_spmd(nc, in_maps, core_ids=list(range(N_CORES)), trace=trace)
    y = np.concatenate([r["y"] for r in res.results], axis=0)
    ld = np.concatenate([r["log_det"] for r in res.results], axis=0)
    return (y, ld), res


def kernel(**inputs):
    out, _ = _run(inputs, trace=False)
    return out


# revision 17
# speedup vs baseline: 1.0584x; 1.0572x over previous
"""Trainium2 Bass kernel for ConditionalAffineCoupling (dense MLP coupling layer).

Computation (per row of the batch):
    x_masked  = x[:, 0::2]            # [B, 256]
    x_unmask  = x[:, 1::2]            # [B, 256]
    st_in     = concat([x_masked, context], -1)      # [B, 384]
    h         = relu(st_in @ W1 + b1)                # [B, 2048]
    st        = h @ W2 + b2                          # [B, 512]
    s         = tanh(st[:, :256]) * 5.0
    y_odd     = x_unmask * exp(s) + st[:, 256:]
    y         = x with odd cols replaced by y_odd
    log_det   = s.sum(-1)

Strategy: data-parallel over 8 NeuronCores (batch sharded, weights
replicated). Per core, 512-row tiles:
  - PE transposes build st_in^T in SBUF (fp32, exact),
  - layer 1 in float32r (full PE rate at N=512): h^T = W1^T-slices
    (stationary) x st_in^T, bias+ReLU fused on ScalarE during the
    PSUM->SBUF drain,
  - layer 2 produces st in natural [row, feature] layout: lhsT = h^T
    sub-tile, rhs = W2 (resident); b2 folded in via a K=1 matmul
    against a constant-ones row,
  - epilogue: tanh on ScalarE with accum_out accumulating
    sum(tanh) per row (log_det = 5 * that), exp(5*tanh) on ScalarE,
    then two in-place DVE ops write y's odd columns straight into the
    x tile, which is DMAed out as full y rows.
"""

import sys

sys.path.insert(0, "/opt/trn_rl_repo")

import numpy as np

import concourse.mybir as mybir
import concourse.tile as tile
from concourse import bacc
from concourse.bass_utils import run_bass_kernel_spmd
from concourse.masks import make_identity

B, DIM, CTX, HID = 131072, 512, 128, 2048
DU = DIM // 2  # 256
S_MAX = 5.0
N_CORES = 8
BS = B // N_CORES  # 16384 rows per core
TR = 512  # rows per tile
NT = BS // TR  # 32 tiles
SUB = TR // 128  # 4 sub-tiles of 128 rows
KT1 = (DIM // 2 + CTX) // 128  # 3 contraction tiles, layer 1
KT2 = HID // 128  # 16 contraction tiles, layer 2

F32 = mybir.dt.float32
F32R = mybir.dt.float32r
AF = mybir.ActivationFunctionType

_CACHE = {}


def build_nc(repeat=1, ablate=0):
    """ablate (timing experiments only; breaks correctness):
    1: no y DMA; 2: + no tanh/exp/mul/add/ld; 3: + no relu drains;
    4: + no transposes/stT copies; 5: + no x/ctx input DMAs."""
    nc = bacc.Bacc("TRN2", target_bir_lowering=False, debug=False)
    x_d = nc.dram_tensor("x", [BS, DIM], F32, kind="ExternalInput")
    c_d = nc.dram_tensor("context", [BS, CTX], F32, kind="ExternalInput")
    w1_d = nc.dram_tensor("W1", [DIM // 2 + CTX, HID], F32, kind="ExternalInput")
    b1_d = nc.dram_tensor("b1", [HID], F32, kind="ExternalInput")
    w2_d = nc.dram_tensor("W2", [HID, 2 * DU], F32, kind="ExternalInput")
    b2_d = nc.dram_tensor("b2", [2 * DU], F32, kind="ExternalInput")
    y_d = nc.dram_tensor("y", [BS, DIM], F32, kind="ExternalOutput")
    ld_d = nc.dram_tensor("log_det", [BS], F32, kind="ExternalOutput")

    # [NT, 128, SUB, DIM]: row r = (t*SUB + j)*128 + p
    x4 = x_d.ap().rearrange("(t j p) d -> t p j d", p=128, j=SUB)
    c4 = c_d.ap().rearrange("(t j p) d -> t p j d", p=128, j=SUB)
    y4 = y_d.ap().rearrange("(t j p) d -> t p j d", p=128, j=SUB)
    # weights: contraction index k = kt*128 + p
    w1r = w1_d.ap().bitcast(F32R).rearrange("(kt p) n -> p kt n", p=128)
    w2r = w2_d.ap().bitcast(F32R).rearrange("(kt p) n -> p kt n", p=128)
    b1v = b1_d.ap().rearrange("(t p) -> p t", p=128)  # [128, KT2]
    b2r = b2_d.ap().bitcast(F32R).rearrange("(o n) -> o n", o=1)  # [1, 512]
    ld2 = ld_d.ap().rearrange("(c p) -> c p", p=128)  # [128, 128]

    with tile.TileContext(nc) as tc:
        with (
            tc.tile_pool(name="const", bufs=1) as constp,
            tc.tile_pool(name="wpool", bufs=1) as wp,
            tc.tile_pool(name="xpool", bufs=3) as xp,
            tc.tile_pool(name="stpool", bufs=2) as stp,
            tc.tile_pool(name="hpool", bufs=2) as hp,
            tc.tile_pool(name="epi", bufs=3) as ep,
            tc.tile_pool(name="ldp", bufs=1) as ldp,
            tc.tile_pool(name="pstr", bufs=2, space="PSUM") as pst,
            tc.tile_pool(name="ps1", bufs=3, space="PSUM") as ps1,
            tc.tile_pool(name="ps2", bufs=2, space="PSUM") as ps2,
        ):
            ident = constp.tile([128, 128], F32)
            make_identity(nc, ident[:])
            zeros = constp.tile([128, TR], F32)
            nc.gpsimd.memset(zeros[:], 0.0)

            # weights go via SWDGE (gpsimd) so the sync HWDGE queue is free
            # for the first x/context tiles; b1/b2 are tiny, send first.
            b1sb = wp.tile([128, KT2], F32)
            nc.gpsimd.dma_start(b1sb[:], b1v)
            b2sb = wp.tile([1, 2 * DU], F32R)
            nc.gpsimd.dma_start(b2sb[:], b2r)
            w1sb = wp.tile([128, KT1, HID], F32R)
            nc.gpsimd.dma_start(w1sb[:], w1r)
            w2sb = wp.tile([128, KT2, 2 * DU], F32R)
            nc.gpsimd.dma_start(w2sb[:], w2r)

            stT_pre = hT_pre = None
            if ablate >= 4:
                stT_pre = stp.tile([128, KT1, TR], F32R, name="stT_pre", tag="stT")
                for kt in range(KT1):
                    nc.scalar.copy(stT_pre[:, kt, :], zeros[:])
            if ablate >= 3:
                hT_pre = hp.tile([128, KT2, TR], F32R, name="hT_pre", tag="hT")
                for ht in range(KT2):
                    nc.scalar.copy(hT_pre[:, ht, :], zeros[:])

            def body(rep=0):
                # per-row sum(tanh); column c = global 128-row sub-tile index
                ld_all = ldp.tile(
                    [128, NT * SUB], F32, name=f"ld_all{rep}", tag="ld_all"
                )

                def load_xc(t):
                    xt = xp.tile([128, SUB, DIM], F32, name=f"xt{t}", tag="xt")
                    nc.sync.dma_start(xt[:], x4[t])
                    ct = xp.tile([128, SUB, CTX], F32, name=f"ct{t}", tag="ct")
                    nc.sync.dma_start(ct[:], c4[t])
                    return xt, ct

                def make_tr_ops(t, xt, ct, stT):
                    """Closures emitting one transpose + drain each; st_in^T
                    [384, TR] as [128, KT1, TR] (k = kt*128 + p)."""
                    ops = []
                    for j in range(SUB):
                        xj = xt[:, j].rearrange("p (d two) -> p two d", two=2)
                        for ft in range(2):

                            def op(j=j, ft=ft, xj=xj):
                                tp = pst.tile(
                                    [128, 128], F32, name=f"tp{t}_{j}_{ft}", tag="tp"
                                )
                                nc.tensor.transpose(
                                    tp[:],
                                    xj[:, 0, ft * 128 : (ft + 1) * 128],
                                    ident[:],
                                )
                                nc.vector.tensor_copy(
                                    stT[:, ft, j * 128 : (j + 1) * 128], tp[:]
                                )

                            ops.append(op)

                        def opc(j=j):
                            tpc = pst.tile(
                                [128, 128], F32, name=f"tpc{t}_{j}", tag="tp"
                            )
                            nc.tensor.transpose(tpc[:], ct[:, j], ident[:])
                            nc.vector.tensor_copy(
                                stT[:, 2, j * 128 : (j + 1) * 128], tpc[:]
                            )

                        ops.append(opc)
                    return ops

                # prologue: first tile's inputs + transposes
                if ablate < 5:
                    xt, ct = load_xc(0)
                if ablate < 4:
                    stT = stp.tile([128, KT1, TR], F32R, name="stT0", tag="stT")
                    for op in make_tr_ops(0, xt, ct, stT):
                        op()
                else:
                    stT = stT_pre

                for t in range(NT):
                    # prefetch next tile's inputs
                    if ablate < 5 and t + 1 < NT:
                        xt_nxt, ct_nxt = load_xc(t + 1)

                    # layer 1: h^T [HID, TR] as [128, KT2, TR]
                    if ablate < 3:
                        hT = hp.tile([128, KT2, TR], F32R, name=f"hT{t}", tag="hT")
                    else:
                        hT = hT_pre
                    for ht in range(KT2):
                        ph = ps1.tile([128, TR], F32, name=f"ph{t}_{ht}", tag="ph")
                        for kt in range(KT1):
                            nc.tensor.matmul(
                                ph[:],
                                w1sb[:, kt, ht * 128 : (ht + 1) * 128],
                                stT[:, kt, :],
                                start=(kt == 0),
                                stop=(kt == KT1 - 1),
                            )
                        # relu(psum + b1) -> f32r, alternating DVE / ACT
                        if ablate < 3:
                            if ht % 2 == 0:
                                nc.vector.scalar_tensor_tensor(
                                    hT[:, ht, :],
                                    ph[:],
                                    b1sb[:, ht : ht + 1],
                                    zeros[:],
                                    op0=mybir.AluOpType.add,
                                    op1=mybir.AluOpType.max,
                                )
                            else:
                                nc.scalar.activation(
                                    hT[:, ht, :],
                                    ph[:],
                                    AF.Relu,
                                    bias=b1sb[:, ht : ht + 1],
                                )

                    # next tile's transposes, interleaved into the L2 MM
                    # stream so PE never waits on the psum->SBUF drains
                    tr_ops = []
                    if ablate < 4 and t + 1 < NT:
                        stT = stp.tile(
                            [128, KT1, TR], F32R, name=f"stT{t + 1}", tag="stT"
                        )
                        tr_ops = make_tr_ops(t + 1, xt_nxt, ct_nxt, stT)
                    mm_i = 0

                    # layer 2 + epilogue per 128-row sub-tile
                    for j in range(SUB):
                        p2 = ps2.tile([128, 2 * DU], F32, name=f"p2_{t}_{j}", tag="p2")
                        nc.tensor.matmul(
                            p2[:], ones_r[:], b2sb[:], start=True, stop=False
                        )
                        for kt in range(KT2):
                            nc.tensor.matmul(
                                p2[:],
                                hT[:, kt, j * 128 : (j + 1) * 128],
                                w2sb[:, kt, :],
                                start=False,
                                stop=(kt == KT2 - 1),
                            )
                            mm_i += 1
                            if mm_i % 5 == 0 and tr_ops:
                                tr_ops.pop(0)()
                        if ablate >= 2:
                            continue
                        tanh_s = ep.tile(
                            [128, DU], F32, name=f"tanh{t}_{j}", tag="tanh"
                        )
                        col = t * SUB + j
                        nc.scalar.activation(
                            tanh_s[:],
                            p2[:, 0:DU],
                            AF.Tanh,
                            accum_out=ld_all[:, col : col + 1],
                        )
                        exp_s = ep.tile([128, DU], F32, name=f"exp{t}_{j}", tag="exp")
                        nc.scalar.activation(exp_s[:], tanh_s[:], AF.Exp, scale=S_MAX)
                        xj = xt[:, j].rearrange("p (d two) -> p two d", two=2)
                        nc.vector.tensor_mul(xj[:, 1, :], xj[:, 1, :], exp_s[:])
                        nc.vector.tensor_add(
                            xj[:, 1, :], xj[:, 1, :], p2[:, DU : 2 * DU]
                        )
                        if ablate >= 1:
                            continue
                        # y out via the ACT HWDGE queue (inputs own sync's)
                        nc.scalar.dma_start(y4[t, :, j], xt[:, j])

                    for op in tr_ops:  # any leftovers
                        op()
                    if ablate < 5:
                        xt, ct = (xt_nxt, ct_nxt) if t + 1 < NT else (None, None)

                    # log_det = 5 * sum(tanh): drain every 8 tiles so the
                    # tail transpose isn't serialized at the very end
                    if ablate < 2 and (t + 1) % 8 == 0:
                        c = (t + 1) // 8 - 1
                        cols = slice(32 * c, 32 * (c + 1))
                        ldps = pst.tile(
                            [32, 128], F32, name=f"ldps{c}", tag="ldps", bufs=1
                        )
                        nc.tensor.transpose(ldps[:], ld_all[:, cols], ident[:])
                        ld_out = ldp.tile(
                            [32, 128], F32, name=f"ld_out{rep}_{c}", tag="ld_out"
                        )
                        nc.scalar.mul(ld_out[:], ldps[:], S_MAX)
                        nc.scalar.dma_start(ld2[cols], ld_out[:])

            if repeat == 1:
                body()
            else:
                with tc.For_i(0, repeat, 1):
                    body()

    nc.compile()
    return nc


def make_in_maps(inputs):
    x = np.ascontiguousarray(np.asarray(inputs["x"], dtype=np.float32))
    ctx = np.ascontiguousarray(np.asarray(inputs["context"], dtype=np.float32))
    w1 = np.ascontiguousarray(np.asarray(inputs["W1"], dtype=np.float32))
    b1 = np.ascontiguousarray(np.asarray(inputs["b1"], dtype=np.float32))
    w2 = np.ascontiguousarray(np.asarray(inputs["W2"], dtype=np.float32))
    b2 = np.ascontiguousarray(np.asarray(inputs["b2"], dtype=np.float32))

    in_maps = []
    for i in range(N_CORES):
        sl = slice(i * BS, (i + 1) * BS)
        in_maps.append(
            {
                "x": np.ascontiguousarray(x[sl]),
                "context": np.ascontiguousarray(ctx[sl]),
                "W1": w1,
                "b1": b1,
                "W2": w2,
                "b2": b2,
            }
        )
    return in_maps


def _run(inputs, trace=False):
    if "nc" not in _CACHE:
        _CACHE["nc"] = build_nc()
    nc = _CACHE["nc"]
    in_maps = make_in_maps(inputs)
    res = run_bass_kernel_spmd(nc, in_maps, core_ids=list(range(N_CORES)), trace=trace)
    y = np.concatenate([r["y"] for r in res.results], axis=0)
    ld = np.concatenate([r["log_det"] for r in res.results], axis=0)
    return (y, ld), res


def kernel(**inputs):
    out, _ = _run(inputs, trace=False)
    return out
